# revision 15
# baseline (speedup 1.0000x reference)
"""CondConv (MoE-routed conv) Trainium2 kernel.

Problem: B=32 samples of [128, 64, 64]; routing = softmax(linear(global-avg-pool));
per-sample conv weights = routing-weighted sum of 8 expert banks [256,128,3,3];
output = VALID 3x3 conv -> [32, 256, 62, 62] fp32.

Strategy: data-parallel over batch across 8 NeuronCores (4 samples/core), expert
bank replicated. Per core:
  - ACT: per-sample Copy with accum_out -> global-sum pool (cast scratch is the
    side product), PSUM drains with 62-of-64 column discard.
  - PE: routing logits (fp32), ones-vector broadcast of routing weights, and the
    conv as implicit GEMM: for each 3x3 tap, a bf16 matmul [K=128ci, M=128co,
    N=512 (8 rows x 64 cols)] accumulating 9 taps in one PSUM bank.
  - DVE: softmax + 8-term expert combine via scalar_tensor_tensor chains (bf16).
Host side pre-shards x (bf16), pre-transposes the expert bank to [ci, e, tap, co]
bf16, and pre-scales routing weights by 1/4096 to fold in the mean.
"""

import sys

import numpy as np
import ml_dtypes

sys.path.insert(0, "/opt/trn_rl_repo")

B, CIN, COUT, KS, E, H, W = 32, 128, 256, 3, 8, 64, 64
NCORES = 8
BL = B // NCORES          # samples per core
HO, WO = H - KS + 1, W - KS + 1   # 62, 62
KK = KS * KS              # 9 taps
PIX = H * W               # 4096
XPAD = 64                 # tail slack: last row-block's shifted reads overrun by <= kx
CO2 = COUT // 2           # 128-wide output-channel chunks

_BF16 = ml_dtypes.bfloat16


def _build_nc(enable_asserts=False):
    import concourse.bacc as bacc
    import concourse.tile as tile
    from concourse import mybir
    from contextlib import ExitStack

    f32 = mybir.dt.float32
    bf16 = mybir.dt.bfloat16
    AF = mybir.ActivationFunctionType
    ALU = mybir.AluOpType

    nc = bacc.Bacc(
        "TRN2",
        target_bir_lowering=False,
        debug=False,
        enable_asserts=enable_asserts,
        num_devices=NCORES,
    )

    x_d = nc.dram_tensor("x", [BL, CIN, PIX], bf16, kind="ExternalInput")
    ew_d = nc.dram_tensor("ew", [CIN, E, KK * COUT], bf16, kind="ExternalInput")
    rw_d = nc.dram_tensor("rw", [CIN, E], f32, kind="ExternalInput")
    rb_d = nc.dram_tensor("rb", [1, E], f32, kind="ExternalInput")
    o_d = nc.dram_tensor("out", [BL, COUT, HO, WO], f32, kind="ExternalOutput")

    with ExitStack() as ctx:
        tc = ctx.enter_context(tile.TileContext(nc))
        singles = ctx.enter_context(tc.tile_pool(name="singles", bufs=1))
        small = ctx.enter_context(tc.tile_pool(name="small", bufs=2))
        cw_pool = ctx.enter_context(tc.tile_pool(name="cw", bufs=2))
        oc_pool = ctx.enter_context(tc.tile_pool(name="ostage", bufs=2))
        ps_pool = ctx.enter_context(tc.tile_pool(name="psum", bufs=4, space="PSUM"))
        ps_small = ctx.enter_context(tc.tile_pool(name="psum_s", bufs=2, space="PSUM"))

        # Resident inputs
        ew_sb = singles.tile([CIN, E, KK * COUT], bf16)
        nc.sync.dma_start(out=ew_sb[:], in_=ew_d.ap())
        x_t = []
        for b in range(BL):
            xt = singles.tile([CIN, PIX], bf16, tag=f"x{b}")
            nc.sync.dma_start(out=xt[:], in_=x_d.ap()[b])
            x_t.append(xt)
        rw_sb = singles.tile([CIN, E], f32)
        nc.sync.dma_start(out=rw_sb[:], in_=rw_d.ap())
        rb_sb = singles.tile([1, E], f32)
        nc.sync.dma_start(out=rb_sb[:], in_=rb_d.ap())

        ones_sb = singles.tile([1, CIN], f32)
        nc.vector.memset(ones_sb[:], 1.0)

        pooled = singles.tile([CIN, BL], f32)
        rw_bc = singles.tile([CIN, BL * E], f32)  # broadcast softmax weights

        # Warmup matmul: advances PE's DMA clock past the rw_sb load so the
        # per-sample fused fp32 matmuls need only one sync wait (TRN2's
        # weight-load instruction has a single wait slot).
        warm_ps = ps_small.tile([E, E], f32, tag="bcps")
        nc.tensor.matmul(warm_ps[:], lhsT=rw_sb[:], rhs=rw_sb[:], start=True, stop=True)

        for b in range(BL):
            # --- routing: global-sum pool via in-place ACT Copy with accum_out
            nc.scalar.activation(
                out=x_t[b][:],
                in_=x_t[b][:],
                func=AF.Copy,
                accum_out=pooled[:, b:b + 1],
            )
            # logits = pooled/PIX @ rw^T + bias  (1/PIX folded into rw host-side)
            lg_ps = ps_small.tile([1, E], f32, tag="lgps")
            nc.tensor.matmul(
                lg_ps[:], lhsT=pooled[:, b:b + 1], rhs=rw_sb[:], start=True, stop=True
            )
            lg = small.tile([1, E], f32, tag="lg")
            nc.vector.tensor_add(lg[:], lg_ps[:], rb_sb[:])
            # softmax over 8 experts on partition 0
            mx = small.tile([1, 1], f32, tag="mx")
            nc.vector.tensor_reduce(mx[:], lg[:], axis=mybir.AxisListType.X, op=ALU.max)
            nc.vector.tensor_scalar_sub(lg[:], lg[:], mx[:])
            ex = small.tile([1, E], f32, tag="ex")
            se = small.tile([1, 1], f32, tag="se")
            nc.scalar.activation(out=ex[:], in_=lg[:], func=AF.Exp, accum_out=se[:])
            rc = small.tile([1, 1], f32, tag="rc")
            nc.vector.reciprocal(rc[:], se[:])
            nc.vector.tensor_scalar_mul(ex[:], ex[:], rc[:])
            # broadcast [1, E] -> [128, E] with a rank-1 ones matmul
            bc_ps = ps_small.tile([CIN, E], f32, tag="bcps")
            nc.tensor.matmul(bc_ps[:], lhsT=ones_sb[:], rhs=ex[:], start=True, stop=True)
            nc.vector.tensor_copy(out=rw_bc[:, b * E:(b + 1) * E], in_=bc_ps[:])

            # --- combine expert weights: cw = sum_e r[b,e] * ew[e]  (bf16, DVE)
            cw = cw_pool.tile([CIN, KK * COUT], bf16)
            nc.vector.tensor_scalar_mul(
                cw[:], ew_sb[:, 0], rw_bc[:, b * E:b * E + 1]
            )
            for e in range(1, E):
                nc.vector.scalar_tensor_tensor(
                    out=cw[:],
                    in0=ew_sb[:, e],
                    scalar=rw_bc[:, b * E + e:b * E + e + 1],
                    in1=cw[:],
                    op0=ALU.mult,
                    op1=ALU.add,
                )

            # --- conv: implicit GEMM over 9 taps, 8-row blocks, 2 co chunks
            for chunk in range(2):
                ost = oc_pool.tile([CO2, HO * WO], f32)
                for rblk in range(8):
                    rows = 8 if rblk < 7 else 6
                    n = rows * WO
                    pt = ps_pool.tile([CO2, 512], f32, tag="conv")
                    for kki in range(KK):
                        ky, kx = divmod(kki, KS)
                        lhsT = cw[:, kki * COUT + chunk * CO2:
                                  kki * COUT + chunk * CO2 + CO2]
                        xv = x_t[b][:].rearrange("p (h w) -> p h w", w=W)
                        r0 = rblk * 8 + ky
                        rhs = xv[:, r0:r0 + rows, kx:kx + WO]
                        nc.tensor.matmul(
                            pt[:, :n],
                            lhsT=lhsT,
                            rhs=rhs,
                            start=(kki == 0),
                            stop=(kki == KK - 1),
                        )
                    nc.scalar.copy(
                        out=ost[:, rblk * 8 * WO:(rblk * 8 + rows) * WO],
                        in_=pt[:, :n],
                    )
                nc.sync.dma_start(
                    out=o_d.ap()[b, chunk * CO2:(chunk + 1) * CO2].rearrange(
                        "c h w -> c (h w)"
                    ),
                    in_=ost[:],
                )
    nc.compile()
    return nc


def _prep_inputs(x, expert_weight, routing_w, routing_b):
    """Host-side shard + layout prep. Returns per-core input maps."""
    x = np.asarray(x, dtype=np.float32)
    ew = np.asarray(expert_weight, dtype=np.float32)
    rw = np.asarray(routing_w, dtype=np.float32)
    rb = np.asarray(routing_b, dtype=np.float32)

    xb = np.ascontiguousarray(x.reshape(B, CIN, PIX)).astype(_BF16)
    # [E, COUT, CIN, KS, KS] -> [CIN, E, KS*KS, COUT] -> flat taps*co
    ewt = np.ascontiguousarray(
        ew.transpose(2, 0, 3, 4, 1).reshape(CIN, E, KK * COUT)
    ).astype(_BF16)
    rwt = np.ascontiguousarray(rw.T / PIX).astype(np.float32)  # [CIN, E], mean folded
    rbv = np.ascontiguousarray(rb.reshape(1, E)).astype(np.float32)

    in_maps = []
    for c in range(NCORES):
        in_maps.append(
            {
                "x": np.ascontiguousarray(xb[c * BL:(c + 1) * BL]),
                "ew": ewt,
                "rw": rwt,
                "rb": rbv,
            }
        )
    return in_maps


_NC_CACHE = {}


def get_nc(enable_asserts=False):
    key = bool(enable_asserts)
    if key not in _NC_CACHE:
        _NC_CACHE[key] = _build_nc(enable_asserts=enable_asserts)
    return _NC_CACHE[key]


def kernel(x, expert_weight, routing_w, routing_b):
    from concourse import bass_utils

    nc = get_nc()
    in_maps = _prep_inputs(x, expert_weight, routing_w, routing_b)
    res = bass_utils.run_bass_kernel_spmd(nc, in_maps, core_ids=list(range(NCORES)))
    out = np.concatenate([r["out"] for r in res.results], axis=0)
    return out.astype(np.float32)
